# revision 29
# baseline (speedup 1.0000x reference)
"""Multi-head attention (B=2,S=2048,D=1024,H=16) on 8 TRN2 NeuronCores.

Sharding: core c handles head-PAIR c (heads 2c, 2c+1) of BOTH batches
(tensor parallel over heads; both batches per core so the per-batch
valid_lens tile counts need no SPMD padding). wq/wk/wv are split
column-wise by pair, wo row-wise. Each core computes partial output
projections outT[b] [D,S]; the host sums the 8 partials per batch,
transposes, and adds bo.

Device layout per core ("T" = [feature, seq] orientation):
  qT[b] = (wq_p^T @ xq_b^T) * 0.125 + bq*0.125   [128, S]
  kT[b] =  wk_p^T @ xk_b^T + bk                  [128, S]
  v[b]  =  xv_b @ wv_p + bv (ones-row matmul)    [S, 128] (+ones col/head)
  per (b, sq-block, key-tile t):
    scoresT(hh) = kT_h[:,t]^T-stat @ qT_h        [128 sk, 512 sq] psum
      (hh=0 on PE rows 0-63, hh=1 rows 64-127 -> concurrent pair)
    pT = exp(scoresT + mask_bias[b][t])          one 1024-wide ACT call
    oT_ext(hh) += [v_h[t] | 1]^T-stat @ pT(hh)   [65, 512] psum
  oT = oT_ext[0:64] * broadcast(1/rowsum);  outT[b] += wo_p^T @ oT

Key-padding mask is exact: bias[t*128+p] = 0 / -30000 from valid_lens;
key tiles beyond nblk_b = ceil(valid_lens[b]/128) are skipped (identical
loop bounds on every core -> SPMD-safe).
"""

import sys

if "/opt/trn_rl_repo" not in sys.path:
    sys.path.insert(0, "/opt/trn_rl_repo")

from contextlib import ExitStack

import numpy as np
import ml_dtypes

from concourse import bass, bacc, mybir
from concourse import tile
from concourse.bass_utils import run_bass_kernel_spmd

BF16 = mybir.dt.bfloat16
F32 = mybir.dt.float32
npbf16 = ml_dtypes.bfloat16

B, S, D, H, DH = 2, 2048, 1024, 16, 64
NCORES = 8
PW = 2 * DH  # 128, head-pair width = per-core projection width
NKC = D // 128  # 8 contraction chunks for projections
NST = S // 128  # 16 key tiles
SQB = 512
NSQB = S // SQB  # 4
NDT = D // 128  # 8 output row-tiles
SCALE = 1.0 / 8.0  # 1/sqrt(DH)
MASK_BIAS = -30000.0


def build_nc(nblks) -> bass.Bass:
    nblk0, nblk1 = nblks
    nc = bacc.Bacc()

    x_d = []
    for b in range(B):
        x_d.append(
            tuple(
                nc.declare_dram_parameter(f"x{n}t{b}", [D, S], BF16, isOutput=False)
                for n in "qkv"
            )
        )
    wq_d = nc.declare_dram_parameter("wq", [128, NKC * PW], BF16, isOutput=False)
    wk_d = nc.declare_dram_parameter("wk", [128, NKC * PW], BF16, isOutput=False)
    wv_d = nc.declare_dram_parameter("wv", [128, NKC * PW], BF16, isOutput=False)
    wo_d = nc.declare_dram_parameter("wo", [128, D], BF16, isOutput=False)
    bqk_d = nc.declare_dram_parameter("bqk", [1, 2 * PW], BF16, isOutput=False)
    bvr_d = nc.declare_dram_parameter("bvr", [1, PW], BF16, isOutput=False)
    mb_d = nc.declare_dram_parameter("mb", [128, B * NST], F32, isOutput=False)
    out_d = nc.declare_dram_parameter("outt", [B * D, S], BF16, isOutput=True)

    Exp = mybir.ActivationFunctionType.Exp

    with tile.TileContext(nc) as tc, ExitStack() as ctx:
        cpool = ctx.enter_context(tc.tile_pool(name="consts", bufs=1))
        xpool = ctx.enter_context(tc.tile_pool(name="xin", bufs=3))
        qkpool = ctx.enter_context(tc.tile_pool(name="qk", bufs=1))
        vpool = ctx.enter_context(tc.tile_pool(name="vsb", bufs=1))
        opool = ctx.enter_context(tc.tile_pool(name="osb", bufs=1))
        ptpool = ctx.enter_context(tc.tile_pool(name="ptp", bufs=6))
        smpool = ctx.enter_context(tc.tile_pool(name="small", bufs=3))
        outpool = ctx.enter_context(tc.tile_pool(name="outsb", bufs=4))
        pp = ctx.enter_context(tc.tile_pool(name="pp", bufs=1, space="PSUM"))
        sc = ctx.enter_context(tc.tile_pool(name="sc", bufs=2, space="PSUM"))
        otpp = ctx.enter_context(tc.tile_pool(name="otp", bufs=3, space="PSUM"))

        # ---- constants / weights ----
        wq_sb = cpool.tile([128, NKC * PW], BF16, tag="wq")
        wk_sb = cpool.tile([128, NKC * PW], BF16, tag="wk")
        wv_sb = cpool.tile([128, NKC * PW], BF16, tag="wv")
        wo_sb = cpool.tile([128, D], BF16, tag="wo")
        bqk_sb = cpool.tile([1, 2 * PW], BF16, tag="bqk")
        bvr_sb = cpool.tile([1, PW], BF16, tag="bvr")
        mb_sb = cpool.tile([128, B, NST], F32, tag="mb")
        ones_sb = cpool.tile([1, S], BF16, tag="ones")

        nc.sync.dma_start(out=wq_sb[:], in_=wq_d[:])
        nc.sync.dma_start(out=wk_sb[:], in_=wk_d[:])
        nc.sync.dma_start(out=wv_sb[:], in_=wv_d[:])
        nc.sync.dma_start(out=wo_sb[:], in_=wo_d[:])
        nc.sync.dma_start(out=bqk_sb[:], in_=bqk_d[:])
        nc.sync.dma_start(out=bvr_sb[:], in_=bvr_d[:])
        nc.sync.dma_start(out=mb_sb[:], in_=mb_d.rearrange("p (b t) -> p b t", b=B))
        nc.gpsimd.memset(ones_sb[:], 1.0)

        # warm-up touch: make ScalarE observe the mb DMA once so Exp
        # activations (single sync-wait slot) only ever wait on PE.
        mbtouch = cpool.tile([128, B, NST], F32, tag="mbtouch")
        nc.scalar.copy(mbtouch[:], mb_sb[:])

        qt_sb = qkpool.tile([128, B, S], BF16, tag="qt")
        kt_sb = qkpool.tile([128, B, S], BF16, tag="kt")
        # v with an extra ones column per head: [sk-part, b, tile, head, dh+1]
        v_sb = vpool.tile([128, B, NST, 2, DH + 1], BF16, tag="v")
        nc.gpsimd.memset(v_sb[:, :, :, :, DH : DH + 1], 1.0)
        ot_sb = opool.tile([128, B, S], BF16, tag="ot")

        def load_x(b, nblk):
            """Stream batch-b transposed activations into 3 shared slots.
            xq is needed for all sq; xk/xv only up to the valid key tiles."""
            sk_max = nblk * 128
            tiles = []
            for d, cols in (
                (x_d[b][0], S),
                (x_d[b][1], sk_max),
                (x_d[b][2], sk_max),
            ):
                t = xpool.tile([128, NKC, S], BF16, tag="xt")
                for c in range(NKC):
                    nc.sync.dma_start(
                        out=t[:, c, 0:cols], in_=d[c * 128 : (c + 1) * 128, 0:cols]
                    )
                tiles.append(t)
            return tiles

        def proj_qk(b, xqt, xkt, nblk):
            nkb = -(-nblk * 128 // SQB)  # key blocks actually needed
            for sb in range(NSQB):
                for w_sb, x_sb, dst, bcol in (
                    (wq_sb, xqt, qt_sb, 0),
                    (wk_sb, xkt, kt_sb, 1),
                ):
                    if bcol == 1 and sb >= nkb:
                        continue
                    ps = pp.tile([128, SQB], F32, tag="mm512")
                    for c in range(NKC):
                        nc.tensor.matmul(
                            ps[:],
                            w_sb[:, c * PW : (c + 1) * PW],
                            x_sb[:, c, sb * SQB : (sb + 1) * SQB],
                            start=(c == 0),
                            stop=False,
                        )
                    nc.tensor.matmul(
                        ps[:],
                        bqk_sb[0:1, bcol * PW : (bcol + 1) * PW],
                        ones_sb[0:1, sb * SQB : (sb + 1) * SQB],
                        start=False,
                        stop=True,
                    )
                    nc.vector.tensor_copy(
                        dst[:, b, sb * SQB : (sb + 1) * SQB], ps[:]
                    )

        def proj_v(b, xvt, nblk):
            for t in range(nblk):
                psv = pp.tile([128, 2, DH], F32, tag="mm512")
                for c in range(NKC):
                    nc.tensor.matmul(
                        psv[:],
                        xvt[:, c, t * 128 : (t + 1) * 128],
                        wv_sb[:, c * PW : (c + 1) * PW],
                        start=(c == 0),
                        stop=False,
                    )
                nc.tensor.matmul(
                    psv[:],
                    ones_sb[0:1, t * 128 : (t + 1) * 128],
                    bvr_sb[0:1, :],
                    start=False,
                    stop=True,
                )
                nc.vector.tensor_copy(v_sb[:, b, t, :, 0:DH], psv[:])

        # ---- attention for one (batch, sq-block); the core's head pair
        # runs concurrently via PE row groups (hh=0 rows 0-63, hh=1
        # rows 64-127). One 1024-wide exp covers both heads.
        def attention(b, sqb, nblk):
            sq0 = sqb * SQB
            otp0 = otpp.tile([DH + 1, SQB], F32, tag="otp")
            otp1 = otpp.tile([DH + 1, SQB], F32, tag="otp")
            for t in range(nblk):
                scp = sc.tile([128, 2, SQB], F32, tag="scores")
                nc.tensor.matmul(
                    scp[:, 0, :],
                    kt_sb[0:64, b, t * 128 : (t + 1) * 128],
                    qt_sb[0:64, b, sq0 : sq0 + SQB],
                    start=True,
                    stop=True,
                )
                nc.tensor.matmul(
                    scp[:, 1, :],
                    kt_sb[64:128, b, t * 128 : (t + 1) * 128],
                    qt_sb[64:128, b, sq0 : sq0 + SQB],
                    start=True,
                    stop=True,
                )
                pt = ptpool.tile([128, 2, SQB], BF16, tag="pt")
                nc.scalar.activation(
                    pt[:], scp[:], Exp, bias=mb_sb[:, b, t : t + 1], scale=1.0
                )
                nc.tensor.matmul(
                    otp0[:],
                    v_sb[:, b, t, 0, :],
                    pt[:, 0, :],
                    start=(t == 0),
                    stop=(t == nblk - 1),
                )
                nc.tensor.matmul(
                    otp1[:],
                    v_sb[:, b, t, 1, :],
                    pt[:, 1, :],
                    start=(t == 0),
                    stop=(t == nblk - 1),
                )
            # softmax denominators: row 64 of otp (approx-recip needs a
            # partition-0 source, so stage through a small copy)
            for hh, otp in ((0, otp0), (1, otp1)):
                p0 = 64 * hh
                rs = smpool.tile([1, SQB], F32, tag="rs")
                nc.vector.tensor_copy(rs[:], otp[DH : DH + 1, :])
                recip = smpool.tile([1, SQB], F32, tag="recip")
                nc.vector.reciprocal_approx_fast(recip[:], rs[:])
                bcast = smpool.tile([64, SQB], F32, tag="bcast")
                nc.gpsimd.partition_broadcast(bcast[:], recip[:])
                nc.vector.tensor_mul(
                    ot_sb[p0 : p0 + 64, b, sq0 : sq0 + SQB],
                    otp[0:DH, :],
                    bcast[:],
                )

        # ---- fused partial output projection for one (batch, sq-block)
        def outproj(b, sqb):
            sq0 = sqb * SQB
            for dt in range(NDT):
                pso = pp.tile([128, SQB], F32, tag="mm512")
                nc.tensor.matmul(
                    pso[:],
                    wo_sb[:, dt * 128 : (dt + 1) * 128],
                    ot_sb[:, b, sq0 : sq0 + SQB],
                    start=True,
                    stop=True,
                )
                osb = outpool.tile([128, SQB], BF16, tag="outsb")
                nc.vector.tensor_copy(osb[:], pso[:])
                nc.sync.dma_start(
                    out=out_d[
                        b * D + dt * 128 : b * D + (dt + 1) * 128, sq0 : sq0 + SQB
                    ],
                    in_=osb[:],
                )

        # ---- schedule: batch-1 loads/projections hide under batch-0
        # attention (which is ScalarE-limited)
        nblks_ = (nblk0, nblk1)
        xq0, xk0, xv0 = load_x(0, nblk0)
        proj_qk(0, xq0, xk0, nblk0)
        proj_v(0, xv0, nblk0)
        attention(0, 0, nblk0)
        xq1, xk1, xv1 = load_x(1, nblk1)
        proj_qk(1, xq1, xk1, nblk1)
        proj_v(1, xv1, nblk1)
        attention(1, 0, nblks_[1])
        outproj(0, 0)
        outproj(1, 0)
        for sqb in range(1, NSQB):
            for b in range(B):
                attention(b, sqb, nblks_[b])
                outproj(b, sqb)

    nc.compile()
    return nc


def _chunk_rows(w: np.ndarray, nchunk: int) -> np.ndarray:
    """[nchunk*128, C] -> [128, nchunk*C] with chunk-major columns."""
    c = w.shape[1]
    return np.ascontiguousarray(
        w.reshape(nchunk, 128, c).transpose(1, 0, 2).reshape(128, nchunk * c)
    )


def make_inmaps(inputs: dict):
    xq = np.asarray(inputs["xq"], np.float32)
    xk = np.asarray(inputs["xk"], np.float32)
    xv = np.asarray(inputs["xv"], np.float32)
    wq = np.asarray(inputs["wq"], np.float32)
    bq = np.asarray(inputs["bq"], np.float32)
    wk = np.asarray(inputs["wk"], np.float32)
    bk = np.asarray(inputs["bk"], np.float32)
    wv = np.asarray(inputs["wv"], np.float32)
    bv = np.asarray(inputs["bv"], np.float32)
    wo = np.asarray(inputs["wo"], np.float32)
    valid_lens = np.asarray(inputs["valid_lens"], np.int64)

    nblks = tuple(
        int(min(NST, max(1, -(-int(valid_lens[b]) // 128)))) for b in range(B)
    )

    # shared per-batch transposed activations (bf16)
    xts = {}
    for b in range(B):
        for n, a in (("q", xq), ("k", xk), ("v", xv)):
            xts[f"x{n}t{b}"] = np.ascontiguousarray(a[b].T).astype(npbf16)

    # mask bias columns [128, B*NST]
    mbs = []
    for b in range(B):
        bias = np.where(np.arange(S) < int(valid_lens[b]), 0.0, MASK_BIAS).astype(
            np.float32
        )
        mbs.append(bias.reshape(NST, 128).T)
    mb = np.ascontiguousarray(np.concatenate(mbs, axis=1))

    in_maps = []
    for c in range(NCORES):
        sl = slice(c * PW, (c + 1) * PW)
        in_maps.append(
            {
                **xts,
                "wq": _chunk_rows(wq[:, sl] * SCALE, NKC).astype(npbf16),
                "wk": _chunk_rows(wk[:, sl], NKC).astype(npbf16),
                "wv": _chunk_rows(wv[:, sl], NKC).astype(npbf16),
                "wo": np.ascontiguousarray(wo[sl, :]).astype(npbf16),
                "bqk": np.concatenate([bq[sl] * SCALE, bk[sl]])[None, :].astype(
                    npbf16
                ),
                "bvr": np.ascontiguousarray(bv[sl][None, :]).astype(npbf16),
                "mb": mb,
            }
        )
    return in_maps, nblks


def assemble(results, inputs) -> np.ndarray:
    bo = np.asarray(inputs["bo"], np.float32)
    out = np.zeros((B, S, D), np.float32)
    for c in range(NCORES):
        part = np.asarray(results[c]["outt"], np.float32).reshape(B, D, S)
        for b in range(B):
            out[b] += part[b].T
    out += bo[None, None, :]
    return out


def kernel(**inputs) -> np.ndarray:
    in_maps, nblks = make_inmaps(inputs)
    nc = build_nc(nblks)
    res = run_bass_kernel_spmd(nc, in_maps, core_ids=list(range(NCORES)))
    return assemble(res.results, inputs)


if __name__ == "__main__":
    import reference

    inputs = reference.setup_inputs()
    out = kernel(**{k: np.asarray(v) for k, v in inputs.items()})
    exp = np.asarray(reference.reference(**inputs))
    err = np.linalg.norm(out - exp) / np.linalg.norm(exp)
    print("Relative error:", err)


# revision 30
# speedup vs baseline: 1.0626x; 1.0626x over previous
"""Multi-head attention (B=2,S=2048,D=1024,H=16) on 8 TRN2 NeuronCores.

Sharding: core c handles head-PAIR c (heads 2c, 2c+1) of BOTH batches
(tensor parallel over heads; both batches per core so the per-batch
valid_lens tile counts need no SPMD padding). wq/wk/wv are split
column-wise by pair, wo row-wise. Each core computes partial output
projections outT[b] [D,S]; the host sums the 8 partials per batch,
transposes, and adds bo.

Device layout per core ("T" = [feature, seq] orientation):
  qT[b] = (wq_p^T @ xq_b^T) * 0.125 + bq*0.125   [128, S]
  kT[b] =  wk_p^T @ xk_b^T + bk                  [128, S]
  v[b]  =  xv_b @ wv_p + bv (ones-row matmul)    [S, 128] (+ones col/head)
  per (b, sq-block, key-tile t):
    scoresT(hh) = kT_h[:,t]^T-stat @ qT_h        [128 sk, 512 sq] psum
      (hh=0 on PE rows 0-63, hh=1 rows 64-127 -> concurrent pair)
    pT = exp(scoresT + mask_bias[b][t])          one 1024-wide ACT call
    oT_ext(hh) += [v_h[t] | 1]^T-stat @ pT(hh)   [65, 512] psum
  oT = oT_ext[0:64] * broadcast(1/rowsum);  outT[b] += wo_p^T @ oT

Key-padding mask is exact: bias[t*128+p] = 0 / -30000 from valid_lens;
key tiles beyond nblk_b = ceil(valid_lens[b]/128) are skipped (identical
loop bounds on every core -> SPMD-safe).
"""

import sys

if "/opt/trn_rl_repo" not in sys.path:
    sys.path.insert(0, "/opt/trn_rl_repo")

from contextlib import ExitStack

import numpy as np
import ml_dtypes

from concourse import bass, bacc, mybir
from concourse import tile
from concourse.bass_utils import run_bass_kernel_spmd

BF16 = mybir.dt.bfloat16
F32 = mybir.dt.float32
npbf16 = ml_dtypes.bfloat16

B, S, D, H, DH = 2, 2048, 1024, 16, 64
NCORES = 8
PW = 2 * DH  # 128, head-pair width = per-core projection width
NKC = D // 128  # 8 contraction chunks for projections
NST = S // 128  # 16 key tiles
SQB = 512
NSQB = S // SQB  # 4
NDT = D // 128  # 8 output row-tiles
SCALE = 1.0 / 8.0  # 1/sqrt(DH)
MASK_BIAS = -30000.0


def build_nc(nblks) -> bass.Bass:
    nblk0, nblk1 = nblks
    nc = bacc.Bacc()

    x_d = []
    for b in range(B):
        x_d.append(
            tuple(
                nc.declare_dram_parameter(f"x{n}t{b}", [D, S], BF16, isOutput=False)
                for n in "qkv"
            )
        )
    wq_d = nc.declare_dram_parameter("wq", [128, NKC * PW], BF16, isOutput=False)
    wk_d = nc.declare_dram_parameter("wk", [128, NKC * PW], BF16, isOutput=False)
    wv_d = nc.declare_dram_parameter("wv", [128, NKC * PW], BF16, isOutput=False)
    wo_d = nc.declare_dram_parameter("wo", [128, D], BF16, isOutput=False)
    bqk_d = nc.declare_dram_parameter("bqk", [1, 2 * PW], BF16, isOutput=False)
    bvr_d = nc.declare_dram_parameter("bvr", [1, PW], BF16, isOutput=False)
    mb_d = nc.declare_dram_parameter("mb", [128, B * NST], F32, isOutput=False)
    out_d = nc.declare_dram_parameter("outt", [B * D, S], BF16, isOutput=True)

    Exp = mybir.ActivationFunctionType.Exp

    with tile.TileContext(nc) as tc, ExitStack() as ctx:
        cpool = ctx.enter_context(tc.tile_pool(name="consts", bufs=1))
        xpool = ctx.enter_context(tc.tile_pool(name="xin", bufs=3))
        qkpool = ctx.enter_context(tc.tile_pool(name="qk", bufs=1))
        vpool = ctx.enter_context(tc.tile_pool(name="vsb", bufs=1))
        opool = ctx.enter_context(tc.tile_pool(name="osb", bufs=1))
        ptpool = ctx.enter_context(tc.tile_pool(name="ptp", bufs=6))
        smpool = ctx.enter_context(tc.tile_pool(name="small", bufs=3))
        outpool = ctx.enter_context(tc.tile_pool(name="outsb", bufs=4))
        pp = ctx.enter_context(tc.tile_pool(name="pp", bufs=2, space="PSUM"))
        sc = ctx.enter_context(tc.tile_pool(name="sc", bufs=2, space="PSUM"))
        otpp = ctx.enter_context(tc.tile_pool(name="otp", bufs=2, space="PSUM"))

        # ---- constants / weights ----
        wq_sb = cpool.tile([128, NKC * PW], BF16, tag="wq")
        wk_sb = cpool.tile([128, NKC * PW], BF16, tag="wk")
        wv_sb = cpool.tile([128, NKC * PW], BF16, tag="wv")
        wo_sb = cpool.tile([128, D], BF16, tag="wo")
        bqk_sb = cpool.tile([1, 2 * PW], BF16, tag="bqk")
        bvr_sb = cpool.tile([1, PW], BF16, tag="bvr")
        mb_sb = cpool.tile([128, B, NST], F32, tag="mb")
        ones_sb = cpool.tile([1, S], BF16, tag="ones")

        nc.sync.dma_start(out=wq_sb[:], in_=wq_d[:])
        nc.sync.dma_start(out=wk_sb[:], in_=wk_d[:])
        nc.sync.dma_start(out=wv_sb[:], in_=wv_d[:])
        nc.sync.dma_start(out=wo_sb[:], in_=wo_d[:])
        nc.sync.dma_start(out=bqk_sb[:], in_=bqk_d[:])
        nc.sync.dma_start(out=bvr_sb[:], in_=bvr_d[:])
        nc.sync.dma_start(out=mb_sb[:], in_=mb_d.rearrange("p (b t) -> p b t", b=B))
        nc.gpsimd.memset(ones_sb[:], 1.0)

        # warm-up touch: make ScalarE observe the mb DMA once so Exp
        # activations (single sync-wait slot) only ever wait on PE.
        mbtouch = cpool.tile([128, B, NST], F32, tag="mbtouch")
        nc.scalar.copy(mbtouch[:], mb_sb[:])

        qt_sb = qkpool.tile([128, B, S], BF16, tag="qt")
        kt_sb = qkpool.tile([128, B, S], BF16, tag="kt")
        # v with an extra ones column per head: [sk-part, b, tile, head, dh+1]
        v_sb = vpool.tile([128, B, NST, 2, DH + 1], BF16, tag="v")
        nc.gpsimd.memset(v_sb[:, :, :, :, DH : DH + 1], 1.0)
        ot_sb = opool.tile([128, B, S], BF16, tag="ot")

        def load_x(b, nblk):
            """Stream batch-b transposed activations into 3 shared slots.
            xq is needed for all sq; xk/xv only up to the valid key tiles."""
            sk_max = nblk * 128
            tiles = []
            for d, cols in (
                (x_d[b][0], S),
                (x_d[b][1], sk_max),
                (x_d[b][2], sk_max),
            ):
                t = xpool.tile([128, NKC, S], BF16, tag="xt")
                for c in range(NKC):
                    nc.sync.dma_start(
                        out=t[:, c, 0:cols], in_=d[c * 128 : (c + 1) * 128, 0:cols]
                    )
                tiles.append(t)
            return tiles

        def proj_qk(b, xqt, xkt, nblk):
            nkb = -(-nblk * 128 // SQB)  # key blocks actually needed
            for sb in range(NSQB):
                for w_sb, x_sb, dst, bcol in (
                    (wq_sb, xqt, qt_sb, 0),
                    (wk_sb, xkt, kt_sb, 1),
                ):
                    if bcol == 1 and sb >= nkb:
                        continue
                    ps = pp.tile([128, SQB], F32, tag="mm512")
                    for c in range(NKC):
                        nc.tensor.matmul(
                            ps[:],
                            w_sb[:, c * PW : (c + 1) * PW],
                            x_sb[:, c, sb * SQB : (sb + 1) * SQB],
                            start=(c == 0),
                            stop=False,
                        )
                    nc.tensor.matmul(
                        ps[:],
                        bqk_sb[0:1, bcol * PW : (bcol + 1) * PW],
                        ones_sb[0:1, sb * SQB : (sb + 1) * SQB],
                        start=False,
                        stop=True,
                    )
                    nc.vector.tensor_copy(
                        dst[:, b, sb * SQB : (sb + 1) * SQB], ps[:]
                    )

        def proj_v(b, xvt, nblk):
            for t in range(nblk):
                psv = pp.tile([128, 2, DH], F32, tag="mm512")
                for c in range(NKC):
                    nc.tensor.matmul(
                        psv[:],
                        xvt[:, c, t * 128 : (t + 1) * 128],
                        wv_sb[:, c * PW : (c + 1) * PW],
                        start=(c == 0),
                        stop=False,
                    )
                nc.tensor.matmul(
                    psv[:],
                    ones_sb[0:1, t * 128 : (t + 1) * 128],
                    bvr_sb[0:1, :],
                    start=False,
                    stop=True,
                )
                nc.vector.tensor_copy(v_sb[:, b, t, :, 0:DH], psv[:])

        # ---- attention for one (batch, sq-block); the core's head pair
        # runs concurrently via PE row groups (hh=0 rows 0-63, hh=1
        # rows 64-127). One 1024-wide exp covers both heads.
        def attention(b, sqb, nblk):
            sq0 = sqb * SQB
            otp0 = otpp.tile([DH + 1, SQB], F32, tag="otp")
            otp1 = otpp.tile([DH + 1, SQB], F32, tag="otp")
            for t in range(nblk):
                scp = sc.tile([128, 2, SQB], F32, tag="scores")
                nc.tensor.matmul(
                    scp[:, 0, :],
                    kt_sb[0:64, b, t * 128 : (t + 1) * 128],
                    qt_sb[0:64, b, sq0 : sq0 + SQB],
                    start=True,
                    stop=True,
                )
                nc.tensor.matmul(
                    scp[:, 1, :],
                    kt_sb[64:128, b, t * 128 : (t + 1) * 128],
                    qt_sb[64:128, b, sq0 : sq0 + SQB],
                    start=True,
                    stop=True,
                )
                pt = ptpool.tile([128, 2, SQB], BF16, tag="pt")
                nc.scalar.activation(
                    pt[:], scp[:], Exp, bias=mb_sb[:, b, t : t + 1], scale=1.0
                )
                nc.tensor.matmul(
                    otp0[:],
                    v_sb[:, b, t, 0, :],
                    pt[:, 0, :],
                    start=(t == 0),
                    stop=(t == nblk - 1),
                )
                nc.tensor.matmul(
                    otp1[:],
                    v_sb[:, b, t, 1, :],
                    pt[:, 1, :],
                    start=(t == 0),
                    stop=(t == nblk - 1),
                )
            # softmax denominators: row 64 of otp (approx-recip needs a
            # partition-0 source, so stage through a small copy)
            for hh, otp in ((0, otp0), (1, otp1)):
                p0 = 64 * hh
                rs = smpool.tile([1, SQB], F32, tag="rs")
                nc.vector.tensor_copy(rs[:], otp[DH : DH + 1, :])
                recip = smpool.tile([1, SQB], F32, tag="recip")
                nc.vector.reciprocal_approx_fast(recip[:], rs[:])
                bcast = smpool.tile([64, SQB], F32, tag="bcast")
                nc.gpsimd.partition_broadcast(bcast[:], recip[:])
                nc.vector.tensor_mul(
                    ot_sb[p0 : p0 + 64, b, sq0 : sq0 + SQB],
                    otp[0:DH, :],
                    bcast[:],
                )

        # ---- fused partial output projection for one (batch, sq-block)
        def outproj(b, sqb):
            sq0 = sqb * SQB
            for dt in range(NDT):
                pso = pp.tile([128, SQB], F32, tag="mm512")
                nc.tensor.matmul(
                    pso[:],
                    wo_sb[:, dt * 128 : (dt + 1) * 128],
                    ot_sb[:, b, sq0 : sq0 + SQB],
                    start=True,
                    stop=True,
                )
                osb = outpool.tile([128, SQB], BF16, tag="outsb")
                nc.vector.tensor_copy(osb[:], pso[:])
                nc.sync.dma_start(
                    out=out_d[
                        b * D + dt * 128 : b * D + (dt + 1) * 128, sq0 : sq0 + SQB
                    ],
                    in_=osb[:],
                )

        # ---- schedule: batch-1 loads/projections hide under batch-0
        # attention (which is ScalarE-limited)
        nblks_ = (nblk0, nblk1)
        xq0, xk0, xv0 = load_x(0, nblk0)
        proj_qk(0, xq0, xk0, nblk0)
        proj_v(0, xv0, nblk0)
        attention(0, 0, nblk0)
        xq1, xk1, xv1 = load_x(1, nblk1)
        proj_qk(1, xq1, xk1, nblk1)
        proj_v(1, xv1, nblk1)
        attention(1, 0, nblks_[1])
        outproj(0, 0)
        outproj(1, 0)
        for sqb in range(1, NSQB):
            for b in range(B):
                attention(b, sqb, nblks_[b])
                outproj(b, sqb)

    nc.compile()
    return nc


def _chunk_rows(w: np.ndarray, nchunk: int) -> np.ndarray:
    """[nchunk*128, C] -> [128, nchunk*C] with chunk-major columns."""
    c = w.shape[1]
    return np.ascontiguousarray(
        w.reshape(nchunk, 128, c).transpose(1, 0, 2).reshape(128, nchunk * c)
    )


def make_inmaps(inputs: dict):
    xq = np.asarray(inputs["xq"], np.float32)
    xk = np.asarray(inputs["xk"], np.float32)
    xv = np.asarray(inputs["xv"], np.float32)
    wq = np.asarray(inputs["wq"], np.float32)
    bq = np.asarray(inputs["bq"], np.float32)
    wk = np.asarray(inputs["wk"], np.float32)
    bk = np.asarray(inputs["bk"], np.float32)
    wv = np.asarray(inputs["wv"], np.float32)
    bv = np.asarray(inputs["bv"], np.float32)
    wo = np.asarray(inputs["wo"], np.float32)
    valid_lens = np.asarray(inputs["valid_lens"], np.int64)

    nblks = tuple(
        int(min(NST, max(1, -(-int(valid_lens[b]) // 128)))) for b in range(B)
    )

    # shared per-batch transposed activations (bf16)
    xts = {}
    for b in range(B):
        for n, a in (("q", xq), ("k", xk), ("v", xv)):
            xts[f"x{n}t{b}"] = np.ascontiguousarray(a[b].T).astype(npbf16)

    # mask bias columns [128, B*NST]
    mbs = []
    for b in range(B):
        bias = np.where(np.arange(S) < int(valid_lens[b]), 0.0, MASK_BIAS).astype(
            np.float32
        )
        mbs.append(bias.reshape(NST, 128).T)
    mb = np.ascontiguousarray(np.concatenate(mbs, axis=1))

    in_maps = []
    for c in range(NCORES):
        sl = slice(c * PW, (c + 1) * PW)
        in_maps.append(
            {
                **xts,
                "wq": _chunk_rows(wq[:, sl] * SCALE, NKC).astype(npbf16),
                "wk": _chunk_rows(wk[:, sl], NKC).astype(npbf16),
                "wv": _chunk_rows(wv[:, sl], NKC).astype(npbf16),
                "wo": np.ascontiguousarray(wo[sl, :]).astype(npbf16),
                "bqk": np.concatenate([bq[sl] * SCALE, bk[sl]])[None, :].astype(
                    npbf16
                ),
                "bvr": np.ascontiguousarray(bv[sl][None, :]).astype(npbf16),
                "mb": mb,
            }
        )
    return in_maps, nblks


def assemble(results, inputs) -> np.ndarray:
    bo = np.asarray(inputs["bo"], np.float32)
    out = np.zeros((B, S, D), np.float32)
    for c in range(NCORES):
        part = np.asarray(results[c]["outt"], np.float32).reshape(B, D, S)
        for b in range(B):
            out[b] += part[b].T
    out += bo[None, None, :]
    return out


def kernel(**inputs) -> np.ndarray:
    in_maps, nblks = make_inmaps(inputs)
    nc = build_nc(nblks)
    res = run_bass_kernel_spmd(nc, in_maps, core_ids=list(range(NCORES)))
    return assemble(res.results, inputs)


if __name__ == "__main__":
    import reference

    inputs = reference.setup_inputs()
    out = kernel(**{k: np.asarray(v) for k, v in inputs.items()})
    exp = np.asarray(reference.reference(**inputs))
    err = np.linalg.norm(out - exp) / np.linalg.norm(exp)
    print("Relative error:", err)


# revision 31
# speedup vs baseline: 1.0850x; 1.0210x over previous
"""Multi-head attention (B=2,S=2048,D=1024,H=16) on 8 TRN2 NeuronCores.

Sharding: core c handles head-PAIR c (heads 2c, 2c+1) of BOTH batches
(tensor parallel over heads; both batches per core so the per-batch
valid_lens tile counts need no SPMD padding). wq/wk/wv are split
column-wise by pair, wo row-wise. Each core computes partial output
projections outT[b] [D,S]; the host sums the 8 partials per batch,
transposes, and adds bo.

Device layout per core ("T" = [feature, seq] orientation):
  qT[b] = (wq_p^T @ xq_b^T) * 0.125 + bq*0.125   [128, S]
  kT[b] =  wk_p^T @ xk_b^T + bk                  [128, S]
  v[b]  =  xv_b @ wv_p + bv (ones-row matmul)    [S, 128] (+ones col/head)
  per (b, sq-block, key-tile t):
    scoresT(hh) = kT_h[:,t]^T-stat @ qT_h        [128 sk, 512 sq] psum
      (hh=0 on PE rows 0-63, hh=1 rows 64-127 -> concurrent pair)
    pT = exp(scoresT + mask_bias[b][t])          one 1024-wide ACT call
    oT_ext(hh) += [v_h[t] | 1]^T-stat @ pT(hh)   [65, 512] psum
  oT = oT_ext[0:64] * broadcast(1/rowsum);  outT[b] += wo_p^T @ oT

Key-padding mask is exact: bias[t*128+p] = 0 / -30000 from valid_lens;
key tiles beyond nblk_b = ceil(valid_lens[b]/128) are skipped (identical
loop bounds on every core -> SPMD-safe).
"""

import sys

if "/opt/trn_rl_repo" not in sys.path:
    sys.path.insert(0, "/opt/trn_rl_repo")

from contextlib import ExitStack

import numpy as np
import ml_dtypes

from concourse import bass, bacc, mybir
from concourse import tile
from concourse.bass_utils import run_bass_kernel_spmd

BF16 = mybir.dt.bfloat16
F32 = mybir.dt.float32
npbf16 = ml_dtypes.bfloat16

B, S, D, H, DH = 2, 2048, 1024, 16, 64
NCORES = 8
PW = 2 * DH  # 128, head-pair width = per-core projection width
NKC = D // 128  # 8 contraction chunks for projections
NST = S // 128  # 16 key tiles
SQB = 512
NSQB = S // SQB  # 4
NDT = D // 128  # 8 output row-tiles
SCALE = 1.0 / 8.0  # 1/sqrt(DH)
MASK_BIAS = -30000.0


def build_nc(nblks) -> bass.Bass:
    nblk0, nblk1 = nblks
    nc = bacc.Bacc()

    x_d = []
    for b in range(B):
        x_d.append(
            tuple(
                nc.declare_dram_parameter(f"x{n}t{b}", [D, S], BF16, isOutput=False)
                for n in "qkv"
            )
        )
    wq_d = nc.declare_dram_parameter("wq", [128, NKC * PW], BF16, isOutput=False)
    wk_d = nc.declare_dram_parameter("wk", [128, NKC * PW], BF16, isOutput=False)
    wv_d = nc.declare_dram_parameter("wv", [128, NKC * PW], BF16, isOutput=False)
    wo_d = nc.declare_dram_parameter("wo", [128, D], BF16, isOutput=False)
    bqk_d = nc.declare_dram_parameter("bqk", [1, 2 * PW], BF16, isOutput=False)
    bvr_d = nc.declare_dram_parameter("bvr", [1, PW], BF16, isOutput=False)
    mb_d = nc.declare_dram_parameter("mb", [128, B * NST], F32, isOutput=False)
    out_d = nc.declare_dram_parameter("outt", [B * D, S], BF16, isOutput=True)

    Exp = mybir.ActivationFunctionType.Exp

    with tile.TileContext(nc) as tc, ExitStack() as ctx:
        cpool = ctx.enter_context(tc.tile_pool(name="consts", bufs=1))
        xpool = ctx.enter_context(tc.tile_pool(name="xin", bufs=3))
        qkpool = ctx.enter_context(tc.tile_pool(name="qk", bufs=1))
        vpool = ctx.enter_context(tc.tile_pool(name="vsb", bufs=1))
        opool = ctx.enter_context(tc.tile_pool(name="osb", bufs=1))
        ptpool = ctx.enter_context(tc.tile_pool(name="ptp", bufs=6))
        smpool = ctx.enter_context(tc.tile_pool(name="small", bufs=3))
        outpool = ctx.enter_context(tc.tile_pool(name="outsb", bufs=4))
        pp = ctx.enter_context(tc.tile_pool(name="pp", bufs=2, space="PSUM"))
        sc = ctx.enter_context(tc.tile_pool(name="sc", bufs=2, space="PSUM"))
        otpp = ctx.enter_context(tc.tile_pool(name="otp", bufs=2, space="PSUM"))

        # ---- constants / weights ----
        wq_sb = cpool.tile([128, NKC * PW], BF16, tag="wq")
        wk_sb = cpool.tile([128, NKC * PW], BF16, tag="wk")
        wv_sb = cpool.tile([128, NKC * PW], BF16, tag="wv")
        wo_sb = cpool.tile([128, D], BF16, tag="wo")
        bqk_sb = cpool.tile([1, 2 * PW], BF16, tag="bqk")
        bvr_sb = cpool.tile([1, PW], BF16, tag="bvr")
        mb_sb = cpool.tile([128, B, NST], F32, tag="mb")
        ones_sb = cpool.tile([1, S], BF16, tag="ones")

        nc.sync.dma_start(out=wq_sb[:], in_=wq_d[:])
        nc.sync.dma_start(out=wk_sb[:], in_=wk_d[:])
        nc.sync.dma_start(out=wv_sb[:], in_=wv_d[:])
        nc.sync.dma_start(out=wo_sb[:], in_=wo_d[:])
        nc.sync.dma_start(out=bqk_sb[:], in_=bqk_d[:])
        nc.sync.dma_start(out=bvr_sb[:], in_=bvr_d[:])
        nc.sync.dma_start(out=mb_sb[:], in_=mb_d.rearrange("p (b t) -> p b t", b=B))
        nc.gpsimd.memset(ones_sb[:], 1.0)

        # warm-up touch: make ScalarE observe the mb DMA once so Exp
        # activations (single sync-wait slot) only ever wait on PE.
        mbtouch = cpool.tile([128, B, NST], F32, tag="mbtouch")
        nc.scalar.copy(mbtouch[:], mb_sb[:])

        qt_sb = qkpool.tile([128, B, S], BF16, tag="qt")
        kt_sb = qkpool.tile([128, B, S], BF16, tag="kt")
        # v with an extra ones column per head: [sk-part, b, tile, head, dh+1]
        v_sb = vpool.tile([128, B, NST, 2, DH + 1], BF16, tag="v")
        nc.gpsimd.memset(v_sb[:, :, :, :, DH : DH + 1], 1.0)
        ot_sb = opool.tile([128, B, S], BF16, tag="ot")

        def load_x(b, nblk):
            """Stream batch-b transposed activations into 3 shared slots.
            xq is needed for all sq; xk/xv only up to the valid key tiles."""
            sk_max = nblk * 128
            tiles = []
            for d, cols in (
                (x_d[b][0], S),
                (x_d[b][1], sk_max),
                (x_d[b][2], sk_max),
            ):
                t = xpool.tile([128, NKC, S], BF16, tag="xt")
                for c in range(NKC):
                    nc.sync.dma_start(
                        out=t[:, c, 0:cols], in_=d[c * 128 : (c + 1) * 128, 0:cols]
                    )
                tiles.append(t)
            return tiles

        def proj_qk(b, xqt, xkt, nblk, sbs=None, which="qk"):
            nkb = -(-nblk * 128 // SQB)  # key blocks actually needed
            for sb in sbs if sbs is not None else range(NSQB):
                for w_sb, x_sb, dst, bcol in (
                    (wq_sb, xqt, qt_sb, 0),
                    (wk_sb, xkt, kt_sb, 1),
                ):
                    if bcol == 0 and "q" not in which:
                        continue
                    if bcol == 1 and ("k" not in which or sb >= nkb):
                        continue
                    ps = pp.tile([128, SQB], F32, tag="mm512")
                    for c in range(NKC):
                        nc.tensor.matmul(
                            ps[:],
                            w_sb[:, c * PW : (c + 1) * PW],
                            x_sb[:, c, sb * SQB : (sb + 1) * SQB],
                            start=(c == 0),
                            stop=False,
                        )
                    nc.tensor.matmul(
                        ps[:],
                        bqk_sb[0:1, bcol * PW : (bcol + 1) * PW],
                        ones_sb[0:1, sb * SQB : (sb + 1) * SQB],
                        start=False,
                        stop=True,
                    )
                    nc.vector.tensor_copy(
                        dst[:, b, sb * SQB : (sb + 1) * SQB], ps[:]
                    )

        def proj_v(b, xvt, nblk):
            for t in range(nblk):
                psv = pp.tile([128, 2, DH], F32, tag="mm512")
                for c in range(NKC):
                    nc.tensor.matmul(
                        psv[:],
                        xvt[:, c, t * 128 : (t + 1) * 128],
                        wv_sb[:, c * PW : (c + 1) * PW],
                        start=(c == 0),
                        stop=False,
                    )
                nc.tensor.matmul(
                    psv[:],
                    ones_sb[0:1, t * 128 : (t + 1) * 128],
                    bvr_sb[0:1, :],
                    start=False,
                    stop=True,
                )
                nc.vector.tensor_copy(v_sb[:, b, t, :, 0:DH], psv[:])

        # ---- attention for one (batch, sq-block); the core's head pair
        # runs concurrently via PE row groups (hh=0 rows 0-63, hh=1
        # rows 64-127). One 1024-wide exp covers both heads.
        def attention(b, sqb, nblk, inline_v=None):
            sq0 = sqb * SQB
            otp0 = otpp.tile([DH + 1, SQB], F32, tag="otp")
            otp1 = otpp.tile([DH + 1, SQB], F32, tag="otp")
            for t in range(nblk):
                if inline_v is not None:
                    # project v tile t just-in-time (first sq-block only);
                    # scores/exp for tile t proceed in parallel
                    psv = pp.tile([128, 2, DH], F32, tag="mm512")
                    for c in range(NKC):
                        nc.tensor.matmul(
                            psv[:],
                            inline_v[:, c, t * 128 : (t + 1) * 128],
                            wv_sb[:, c * PW : (c + 1) * PW],
                            start=(c == 0),
                            stop=False,
                        )
                    nc.tensor.matmul(
                        psv[:],
                        ones_sb[0:1, t * 128 : (t + 1) * 128],
                        bvr_sb[0:1, :],
                        start=False,
                        stop=True,
                    )
                    nc.vector.tensor_copy(v_sb[:, b, t, :, 0:DH], psv[:])
                scp = sc.tile([128, 2, SQB], F32, tag="scores")
                nc.tensor.matmul(
                    scp[:, 0, :],
                    kt_sb[0:64, b, t * 128 : (t + 1) * 128],
                    qt_sb[0:64, b, sq0 : sq0 + SQB],
                    start=True,
                    stop=True,
                )
                nc.tensor.matmul(
                    scp[:, 1, :],
                    kt_sb[64:128, b, t * 128 : (t + 1) * 128],
                    qt_sb[64:128, b, sq0 : sq0 + SQB],
                    start=True,
                    stop=True,
                )
                pt = ptpool.tile([128, 2, SQB], BF16, tag="pt")
                nc.scalar.activation(
                    pt[:], scp[:], Exp, bias=mb_sb[:, b, t : t + 1], scale=1.0
                )
                nc.tensor.matmul(
                    otp0[:],
                    v_sb[:, b, t, 0, :],
                    pt[:, 0, :],
                    start=(t == 0),
                    stop=(t == nblk - 1),
                )
                nc.tensor.matmul(
                    otp1[:],
                    v_sb[:, b, t, 1, :],
                    pt[:, 1, :],
                    start=(t == 0),
                    stop=(t == nblk - 1),
                )
            # softmax denominators: row 64 of otp (approx-recip needs a
            # partition-0 source, so stage through a small copy)
            for hh, otp in ((0, otp0), (1, otp1)):
                p0 = 64 * hh
                rs = smpool.tile([1, SQB], F32, tag="rs")
                nc.vector.tensor_copy(rs[:], otp[DH : DH + 1, :])
                recip = smpool.tile([1, SQB], F32, tag="recip")
                nc.vector.reciprocal_approx_fast(recip[:], rs[:])
                bcast = smpool.tile([64, SQB], F32, tag="bcast")
                nc.gpsimd.partition_broadcast(bcast[:], recip[:])
                nc.vector.tensor_mul(
                    ot_sb[p0 : p0 + 64, b, sq0 : sq0 + SQB],
                    otp[0:DH, :],
                    bcast[:],
                )

        # ---- fused partial output projection for one (batch, sq-block)
        def outproj(b, sqb):
            sq0 = sqb * SQB
            for dt in range(NDT):
                pso = pp.tile([128, SQB], F32, tag="mm512")
                nc.tensor.matmul(
                    pso[:],
                    wo_sb[:, dt * 128 : (dt + 1) * 128],
                    ot_sb[:, b, sq0 : sq0 + SQB],
                    start=True,
                    stop=True,
                )
                osb = outpool.tile([128, SQB], BF16, tag="outsb")
                nc.vector.tensor_copy(osb[:], pso[:])
                nc.sync.dma_start(
                    out=out_d[
                        b * D + dt * 128 : b * D + (dt + 1) * 128, sq0 : sq0 + SQB
                    ],
                    in_=osb[:],
                )

        # ---- schedule: batch-1 loads/projections hide under batch-0
        # attention (which is ScalarE-limited)
        nblks_ = (nblk0, nblk1)
        xq0, xk0, xv0 = load_x(0, nblk0)
        proj_qk(0, xq0, xk0, nblk0, sbs=[0], which="q")
        proj_qk(0, xq0, xk0, nblk0, which="k")
        attention(0, 0, nblk0, inline_v=xv0)
        proj_qk(0, xq0, xk0, nblk0, sbs=[1, 2, 3], which="q")
        xq1, xk1, xv1 = load_x(1, nblk1)
        proj_qk(1, xq1, xk1, nblk1)
        proj_v(1, xv1, nblk1)
        attention(1, 0, nblks_[1])
        outproj(0, 0)
        outproj(1, 0)
        for sqb in range(1, NSQB):
            for b in range(B):
                attention(b, sqb, nblks_[b])
                outproj(b, sqb)

    nc.compile()
    return nc


def _chunk_rows(w: np.ndarray, nchunk: int) -> np.ndarray:
    """[nchunk*128, C] -> [128, nchunk*C] with chunk-major columns."""
    c = w.shape[1]
    return np.ascontiguousarray(
        w.reshape(nchunk, 128, c).transpose(1, 0, 2).reshape(128, nchunk * c)
    )


def make_inmaps(inputs: dict):
    xq = np.asarray(inputs["xq"], np.float32)
    xk = np.asarray(inputs["xk"], np.float32)
    xv = np.asarray(inputs["xv"], np.float32)
    wq = np.asarray(inputs["wq"], np.float32)
    bq = np.asarray(inputs["bq"], np.float32)
    wk = np.asarray(inputs["wk"], np.float32)
    bk = np.asarray(inputs["bk"], np.float32)
    wv = np.asarray(inputs["wv"], np.float32)
    bv = np.asarray(inputs["bv"], np.float32)
    wo = np.asarray(inputs["wo"], np.float32)
    valid_lens = np.asarray(inputs["valid_lens"], np.int64)

    nblks = tuple(
        int(min(NST, max(1, -(-int(valid_lens[b]) // 128)))) for b in range(B)
    )

    # shared per-batch transposed activations (bf16)
    xts = {}
    for b in range(B):
        for n, a in (("q", xq), ("k", xk), ("v", xv)):
            xts[f"x{n}t{b}"] = np.ascontiguousarray(a[b].T).astype(npbf16)

    # mask bias columns [128, B*NST]
    mbs = []
    for b in range(B):
        bias = np.where(np.arange(S) < int(valid_lens[b]), 0.0, MASK_BIAS).astype(
            np.float32
        )
        mbs.append(bias.reshape(NST, 128).T)
    mb = np.ascontiguousarray(np.concatenate(mbs, axis=1))

    in_maps = []
    for c in range(NCORES):
        sl = slice(c * PW, (c + 1) * PW)
        in_maps.append(
            {
                **xts,
                "wq": _chunk_rows(wq[:, sl] * SCALE, NKC).astype(npbf16),
                "wk": _chunk_rows(wk[:, sl], NKC).astype(npbf16),
                "wv": _chunk_rows(wv[:, sl], NKC).astype(npbf16),
                "wo": np.ascontiguousarray(wo[sl, :]).astype(npbf16),
                "bqk": np.concatenate([bq[sl] * SCALE, bk[sl]])[None, :].astype(
                    npbf16
                ),
                "bvr": np.ascontiguousarray(bv[sl][None, :]).astype(npbf16),
                "mb": mb,
            }
        )
    return in_maps, nblks


def assemble(results, inputs) -> np.ndarray:
    bo = np.asarray(inputs["bo"], np.float32)
    out = np.zeros((B, S, D), np.float32)
    for c in range(NCORES):
        part = np.asarray(results[c]["outt"], np.float32).reshape(B, D, S)
        for b in range(B):
            out[b] += part[b].T
    out += bo[None, None, :]
    return out


def kernel(**inputs) -> np.ndarray:
    in_maps, nblks = make_inmaps(inputs)
    nc = build_nc(nblks)
    res = run_bass_kernel_spmd(nc, in_maps, core_ids=list(range(NCORES)))
    return assemble(res.results, inputs)


if __name__ == "__main__":
    import reference

    inputs = reference.setup_inputs()
    out = kernel(**{k: np.asarray(v) for k, v in inputs.items()})
    exp = np.asarray(reference.reference(**inputs))
    err = np.linalg.norm(out - exp) / np.linalg.norm(exp)
    print("Relative error:", err)
